# revision 15
# baseline (speedup 1.0000x reference)
"""CubicalLayer persistence-diagram gather on 8 Trainium2 NeuronCores.

reference:
    Xflat = X.reshape(-1)                       # 512^3 f32 (512MB)
    dgm_i = Xflat[indices_i].reshape(-1, 2)     # 2 x 4M random gathers
    zero rows whose |death - birth| <= 0

Strategy (memory-bound; the whole cost is one 8M-way random gather from
a 512MB table):
  * Shard Xflat by element range: core c owns 2^24 elems (64MB), streamed
    through SBUF in 8 chunks of [128 x 16384] f32 (8MB).
  * The host bins all 8M indices by 16384-element row (1024 bins per
    core).  On-chip each row is gathered with GPSIMD ap_gather, whose
    measured cost is ~27.5ns per stream index (SBUF read-command
    latency-bound; per-call overhead is negligible), so the only lever
    is total stream length = sum over calls of max-bin-size-in-call.
  * Pass r serves the 8 bins living in partitions {16g+r}; ap_gather
    applies each group's index list to its 16 partitions.  Four passes
    are fused into one ap_gather call (concatenated 16-aligned
    segments) to amortize per-call overhead; pass r's useful rows
    (exactly partitions p%16==r) go straight to DRAM with a
    partition-strided DMA, so no compute-engine merge is needed.
  * To minimize padding, the host deals the globally count-sorted bins
    round-robin to cores (identical per-core size profiles make the
    SPMD-shared per-call num_idxs near-lossless), then assigns them to
    (chunk, partition) slots in sorted octets, permuting the uploaded X
    rows to match.  Indices are np.unique-deduplicated (~3% birthday
    duplicates) and arrive bin-sorted for free.
  * A tiny warm-up ap_gather issued before the first chunk load hides
    the ucode library load + first-call overhead behind the DMA.
  * The host reassembles: bin b's values sit at out[chunk_b, part_b,
    0:count_b]; scatter back through the argsort permutation, expand
    through the dedup inverse, then apply the min-persistence mask.
"""

import contextlib
import ctypes
import sys
import types

import numpy as np

# ---------------------------------------------------------------- patches


def _install_drain_patch():
    """walrus here rejects >1 sem wait on the Tile tail Drain (TPB_CTRL);
    move the waits onto preceding SP nops, one wait each."""
    import concourse.mybir as mybir
    import concourse.tile as _tile
    from concourse.vector_clock import ScopedClock

    if getattr(_tile.TileContext, "_drain_patched", False):
        return

    def _patched(self, tick_clock, wait_clock):
        nc = self.nc
        probe = nc.sync.nop(nofuse=True, hint="drain_wait_probe")
        wait_clock.add_sem_waits(
            probe.ins, ScopedClock({None: tick_clock.global_clock})
        )
        waits = (
            list(probe.ins.sync_info.on_wait or []) if probe.ins.sync_info else []
        )
        if len(waits) > 1:
            probe.ins.sync_info.on_wait = [waits[0]]
            for w in waits[1:]:
                extra = nc.sync.nop(nofuse=True, hint="drain_wait_split")
                extra.ins.sync_info = mybir.SyncInfo(on_wait=[w], on_update=[])
        nc.sync.drain()
        nc.all_engine_barrier()
        assert self.sems is not None
        popped = nc._tile_sem_poison_stack.pop()
        assert popped is self._sem_poison
        nc.clear_and_free_semaphores(list(self.sems.allocated().values()))
        nc.all_engine_barrier()

    _tile.TileContext._drain_and_barrier = _patched
    _tile.TileContext._drain_patched = True


def _install_profile_hook():
    """Register the NTFF profiling hook bass_utils expects under axon so
    BASS_TRACE=1 yields a HW exec time; degrade silently if unavailable."""
    if "antenv.axon_hooks" in sys.modules:
        return
    try:
        lib = ctypes.CDLL("/opt/axon/libaxon_pjrt.so")
        if not hasattr(lib, "axon_start_nrt_profile"):
            return
        lib.axon_start_nrt_profile.argtypes = [
            ctypes.POINTER(ctypes.c_int64),
            ctypes.c_size_t,
        ]
        lib.axon_start_nrt_profile.restype = ctypes.c_int64
        lib.axon_stop_nrt_profile.argtypes = [ctypes.c_char_p]
        lib.axon_stop_nrt_profile.restype = ctypes.c_int64
    except OSError:
        return

    @contextlib.contextmanager
    def _hook(output_dir, device_ids):
        import jax

        jax.devices()
        if device_ids:
            ids = (ctypes.c_int64 * len(device_ids))(*device_ids)
            rc = lib.axon_start_nrt_profile(ids, len(device_ids))
        else:
            rc = lib.axon_start_nrt_profile(None, 0)
        if rc != 0:
            raise RuntimeError(f"axon_start_nrt_profile rc={rc}")
        try:
            yield
        finally:
            n = lib.axon_stop_nrt_profile(str(output_dir).encode())
            print(f"profile: {n} ntff file(s) in {output_dir}", file=sys.stderr)

    mod = types.ModuleType("antenv.axon_hooks")
    mod.get_axon_ntff_profile_hook = lambda: _hook
    mod.set_axon_ntff_profile_hook = lambda h: None
    sys.modules["antenv.axon_hooks"] = mod

    from concourse import bass_utils as bu

    bu.upload_artifacts = lambda tmpdir: "local://" + tmpdir


# ------------------------------------------------------------------ plan


class Plan:
    def __init__(self, n_cores=8, n_chunks=8, rowlen=16384):
        self.n_cores = n_cores
        self.n_chunks = n_chunks  # chunks per core
        self.rowlen = rowlen  # elements per partition-row (one bin)
        self.rows = 128
        self.chunk_elems = self.rows * rowlen
        self.core_elems = self.chunk_elems * n_chunks
        self.total_elems = self.core_elems * n_cores
        self.bins_per_core = self.rows * n_chunks
        self.n_bins = self.bins_per_core * n_cores
        self.calls_per_core = self.bins_per_core // 8  # 8 groups per band
        self.fuse = 2  # passes fused into one ap_gather call
        # idx tile layout: per fused call, sum of ceil16 segments, start 32-aligned
        self.idx_cols_per_call = 256  # covers 2 segments up to 2048 idx each
        self.idx_cols_per_chunk = 8 * self.idx_cols_per_call


def _host_prep(plan: Plan, all_idx: np.ndarray):
    """Bin indices by 16384-row; deal the globally count-sorted bins
    round-robin to cores (every core gets the same size profile, so the
    SPMD-shared per-call num_idxs loses almost nothing), then octet-assign
    each core's bins to (chunk, partition) slots.  Returns per-core idx
    tiles, the shared per-call sizes, and the metadata to permute the X
    upload and reassemble the output."""
    bins = (all_idx >> int(np.log2(plan.rowlen))).astype(np.int32)
    order = np.argsort(bins, kind="stable")
    bins_sorted = bins[order]
    f_sorted = (all_idx[order] & (plan.rowlen - 1)).astype(np.int16)
    counts = np.bincount(bins, minlength=plan.n_bins)
    starts = np.zeros(plan.n_bins, dtype=np.int64)
    np.cumsum(counts[:-1], out=starts[1:])

    grank = np.argsort(-counts, kind="stable")  # global bin ids, desc count
    # global position i -> (core i%8, per-core slot j=i//8); slot j ->
    # call k=j//8, group g=j%8; call k -> chunk k//16, pass k%16,
    # partition 16g + k%16.  num_idxs for call k is shared by all cores =
    # count at global position 64k (the largest in its 64-bin band).
    # segment sizes rounded to 16 so fused-call segment boundaries fall on
    # whole 16-wrapped idx columns
    nk_shared = np.zeros((plan.n_chunks, 16), dtype=np.int64)
    wmax = 0
    for k in range(plan.calls_per_core):
        n_k = max(16, int(-(-int(counts[grank[64 * k]]) // 16) * 16))
        nk_shared[k // 16, k % 16] = n_k
        wmax = max(wmax, n_k)

    idx_tiles = []  # per core: [n_chunks, 128, idx_cols_per_chunk] int16
    core_bins = []  # per core: global bin id for each local row slot
    row_of_bin = np.zeros(plan.n_bins, dtype=np.int64)  # bin -> global row
    for core in range(plan.n_cores):
        mine = grank[core :: plan.n_cores]  # this core's bins, desc count
        it = np.zeros(
            (plan.n_chunks, 128, plan.idx_cols_per_chunk), dtype=np.int16
        )
        cb = np.zeros(plan.bins_per_core, dtype=np.int64)
        for k in range(plan.calls_per_core):
            c, r = k // 16, k % 16
            n_k = int(nk_shared[c, r])
            n16 = n_k // 16
            # fused call q = r // fuse; segment r % fuse starts after the
            # previous segments' columns
            q = r // plan.fuse
            col0 = q * plan.idx_cols_per_call + sum(
                int(nk_shared[c, rr]) // 16
                for rr in range(q * plan.fuse, r)
            )
            assert col0 + n16 <= (q + 1) * plan.idx_cols_per_call
            for g in range(8):
                gb = int(mine[8 * k + g])
                p = 16 * g + r
                cb[c * 128 + p] = gb
                row_of_bin[gb] = (core * plan.n_chunks + c) * 128 + p
                cnt = int(counts[gb])
                stream = np.zeros(n16 * 16, dtype=np.int16)
                stream[:cnt] = f_sorted[starts[gb] : starts[gb] + cnt]
                # stream i -> partition 16g + i%16, column col0 + i//16
                it[c, 16 * g : 16 * g + 16, col0 : col0 + n16] = stream.reshape(
                    n16, 16
                ).T
        idx_tiles.append(it)
        core_bins.append(cb)
    meta = (order, bins_sorted, counts, starts, row_of_bin, wmax)
    return idx_tiles, core_bins, nk_shared, meta


def _build_program(plan: Plan, nk_shared, wmax: int):
    import concourse.mybir as mybir
    from concourse import bacc, tile

    nc = bacc.Bacc()
    xs = nc.declare_dram_parameter(
        "xs", [plan.n_chunks, 128, plan.rowlen], mybir.dt.float32, isOutput=False
    )
    ix = nc.declare_dram_parameter(
        "ix",
        [plan.n_chunks, 128, plan.idx_cols_per_chunk],
        mybir.dt.int16,
        isOutput=False,
    )
    ov = nc.declare_dram_parameter(
        "ov", [plan.n_chunks, 128, wmax], mybir.dt.float32, isOutput=True
    )

    with tile.TileContext(nc) as tc:
        with (
            tc.tile_pool(name="warm", bufs=1) as warm_pool,
            tc.tile_pool(name="chunks", bufs=2) as chunk_pool,
            tc.tile_pool(name="idx", bufs=2) as idx_pool,
            tc.tile_pool(name="gath", bufs=2) as gath_pool,
        ):
            # warm-up: hide library load + first-call overhead behind DMAs
            wd = warm_pool.tile([128, 16], mybir.dt.float32)
            wi = warm_pool.tile([128, 4], mybir.dt.int16)
            wg = warm_pool.tile([128, 4], mybir.dt.float32)
            nc.gpsimd.memset(wd[:], 0.0)
            nc.gpsimd.memset(wi[:], 0)
            nc.gpsimd.ap_gather(
                wg[:], wd[:], wi[:, 0:1], channels=128, num_elems=16, d=1, num_idxs=4
            )
            for c in range(plan.n_chunks):
                chunk_t = chunk_pool.tile([128, plan.rowlen], mybir.dt.float32)
                nc.sync.dma_start(out=chunk_t[:], in_=xs[c])
                idx_t = idx_pool.tile(
                    [128, plan.idx_cols_per_chunk], mybir.dt.int16
                )
                nc.sync.dma_start(out=idx_t[:], in_=ix[c])
                for q in range(16 // plan.fuse):
                    rs = range(q * plan.fuse, (q + 1) * plan.fuse)
                    seg = [int(nk_shared[c, r]) for r in rs]
                    n_q = sum(seg)
                    col0 = q * plan.idx_cols_per_call
                    gath_t = gath_pool.tile([128, plan.fuse * wmax], mybir.dt.float32)
                    nc.gpsimd.ap_gather(
                        gath_t[:, 0:n_q],
                        chunk_t[:],
                        idx_t[:, col0 : col0 + n_q // 16],
                        channels=128,
                        num_elems=plan.rowlen,
                        d=1,
                        num_idxs=n_q,
                    )
                    # merge = partition-strided DMA straight to DRAM: pass
                    # r's useful rows are exactly partitions p%16==r, so no
                    # compute-engine copy is needed (avoids the DVE<->GPSIMD
                    # shared-SBUF-port stall that a copy_predicated merge
                    # costs the gather stream)
                    off = 0
                    for j, r in enumerate(rs):
                        n_k = seg[j]
                        nc.sync.dma_start(
                            out=ov[c, r : 128 : 16, 0:n_k],
                            in_=gath_t[r : 128 : 16, off : off + n_k],
                        )
                        off += n_k
    nc.finalize()
    return nc


def _assemble(plan: Plan, outs, meta, n_out: int) -> np.ndarray:
    """outs: per-core [n_chunks, 128, wmax] f32 -> flat gather result."""
    order, bins_sorted, counts, starts, row_of_bin, wmax = meta
    b = np.stack(outs).reshape(plan.n_cores * plan.n_chunks * 128, wmax)
    col_idx = np.arange(n_out, dtype=np.int64) - starts[bins_sorted]
    vals_sorted = b[row_of_bin[bins_sorted], col_idx]
    result = np.empty(n_out, dtype=np.float32)
    result[order] = vals_sorted
    return result


LAST_RESULT = None  # BassKernelResults of the most recent run (for test harness)


def _run(plan: Plan, X: np.ndarray, all_idx: np.ndarray) -> np.ndarray:
    global LAST_RESULT
    _install_drain_patch()
    _install_profile_hook()
    from concourse.bass_utils import run_bass_kernel_spmd

    idx_tiles, core_bins, nk_shared, meta = _host_prep(plan, all_idx)
    wmax = meta[5]
    nc = _build_program(plan, nk_shared, wmax)

    xflat = np.ascontiguousarray(X).reshape(-1)
    in_maps = []
    xrows = xflat.reshape(plan.n_bins, plan.rowlen)
    for core in range(plan.n_cores):
        # row slot j of this core holds its assigned global bin core_bins[j]
        xs = xrows[core_bins[core]].reshape(plan.n_chunks, 128, plan.rowlen)
        in_maps.append({"xs": xs, "ix": idx_tiles[core]})
    res = run_bass_kernel_spmd(nc, in_maps, list(range(plan.n_cores)))
    LAST_RESULT = res
    outs = [res.results[c]["ov"] for c in range(plan.n_cores)]
    return _assemble(plan, outs, meta, all_idx.size)


def kernel(X: np.ndarray, indices0: np.ndarray, indices1: np.ndarray):
    plan = Plan()
    assert X.size == plan.total_elems, X.shape
    n0 = indices0.size
    all_idx = np.concatenate([indices0, indices1]).astype(np.int64)
    # ~3% of the 8.4M indices are duplicates (birthday paradox on 134M
    # slots); gather each distinct index once.  np.unique also returns the
    # indices sorted, so within-bin offsets arrive sorted for free.
    uniq, inverse = np.unique(all_idx, return_inverse=True)
    gathered = _run(plan, X, uniq)[inverse]

    def _diagram(vals):
        dgm = vals.reshape(-1, 2)
        keep = np.abs(dgm[:, 1] - dgm[:, 0]) > np.float32(0.0)
        return np.where(keep[:, None], dgm, np.float32(0.0))

    return _diagram(gathered[:n0]), _diagram(gathered[n0:])
